# revision 2
# baseline (speedup 1.0000x reference)
"""Cross-attention kernel for Trainium2 (Bass/Tile), data-parallel over batch on 8 cores.

Reference computation (per batch sample b):
    Q = text @ Wq.T + bq          [T, D]
    K = features @ Wk.T + bk      [P, D]
    scores = Q @ K.T / sqrt(D)    [T, P]
    attn = softmax(scores, -1)
    out = attn @ features         [T, D]

Per-core schedule (one batch sample per NeuronCore):
    Phase A: KT[d,p] = sum_x WkT[x,d]*featT[x,p] + bk  -> kt_dram     (d on partitions)
    Phase B: QT[d,t] = sum_x WqT[x,d]*textT[x,t] + bq  -> qt_dram
    Phase C: per 128-row t-tile:
        scores[t,p] = sum_d QT[d,t]*KT[d,p]   (PSUM, 2 halves of 288)
        softmax over free dim (max via DVE, exp via ACT with fused 1/sqrt(D) scale,
        normalization deferred to the output eviction)
        attnT via PE transpose
        out[t,d] = sum_p attnT[p,t]*features[p,d], scaled by 1/rowsum on eviction

All matmuls use float32r (fp32 storage, full PE rate for free-dim >= 256).
"""

import numpy as np

import concourse.bacc as bacc
import concourse.mybir as mybir
import concourse.tile as tile
from concourse.bass_utils import run_bass_kernel_spmd
from concourse.masks import make_identity

F32 = mybir.dt.float32
F32R = mybir.dt.float32r

# Full problem dims (hardcoded per harness contract)
T_FULL, P_FULL, D_FULL, X_FULL = 2048, 576, 4096, 4096
N_CORES = 8


def build_attention_nc(T=T_FULL, P=P_FULL, D=D_FULL, X=X_FULL):
    assert T % 128 == 0 and D % 128 == 0 and X % 128 == 0
    XO, DT, TT = X // 128, D // 128, T // 128
    PC = -(-P // 128)              # p-chunks for the attended contraction
    P_LAST = P - (PC - 1) * 128
    SCH = P // 2                   # scores half width (288 for P=576); >=256 keeps f32r fast
    assert P % 2 == 0 and SCH <= 512
    TCB = min(1024, T)             # phase-B resident textT chunk
    NTCB = T // TCB
    NB = min(512, TCB)             # phase-B psum free width
    DC = min(512, D)               # attended d chunk
    NDC = D // DC
    scale = 1.0 / float(np.sqrt(D))

    nc = bacc.Bacc()
    textT = nc.dram_tensor("textT", [X, T], F32R, kind="ExternalInput")
    featT = nc.dram_tensor("featT", [X, P], F32R, kind="ExternalInput")
    feat = nc.dram_tensor("feat", [P, D], F32R, kind="ExternalInput")
    wq = nc.dram_tensor("wq", [DT, 128, XO, 128], F32R, kind="ExternalInput")
    wk = nc.dram_tensor("wk", [DT, 128, XO, 128], F32R, kind="ExternalInput")
    bq = nc.dram_tensor("bq", [128, DT], F32, kind="ExternalInput")
    bk = nc.dram_tensor("bk", [128, DT], F32, kind="ExternalInput")
    out = nc.dram_tensor("out", [T, D], F32, kind="ExternalOutput")
    qt_d = nc.dram_tensor("qt_tmp", [DT, 128, T], F32R)
    kt_d = nc.dram_tensor("kt_tmp", [DT, 128, P], F32R)

    textT_v = textT.rearrange("(xo p) t -> p xo t", p=128)
    featT_v = featT.rearrange("(xo p) q -> p xo q", p=128)
    out_v = out.rearrange("(tt p) d -> p tt d", p=128)

    AX = mybir.AxisListType.X
    ALU = mybir.AluOpType
    EXP = mybir.ActivationFunctionType.Exp

    with tile.TileContext(nc) as tc:
        with (
            tc.tile_pool(name="psum", bufs=8, space="PSUM") as psum,
            tc.tile_pool(name="const", bufs=1) as const,
        ):
            ident = const.tile([128, 128], F32)
            make_identity(nc, ident[:])
            bq_sb = const.tile([128, DT], F32, tag="bq")
            nc.sync.dma_start(bq_sb[:], bq[:])
            bk_sb = const.tile([128, DT], F32, tag="bk")
            nc.sync.dma_start(bk_sb[:], bk[:])

            # ---------- Phase A: KT -> kt_dram ----------
            with (
                tc.tile_pool(name="a_w", bufs=2) as a_w,
                tc.tile_pool(name="a_rhs", bufs=1) as a_rhs,
                tc.tile_pool(name="a_out", bufs=3) as a_out,
            ):
                ft_sb = a_rhs.tile([128, XO, P], F32R)
                nc.sync.dma_start(ft_sb[:], featT_v[:])
                for dt in range(DT):
                    w_sb = a_w.tile([128, XO, 128], F32R, tag="aw")
                    nc.sync.dma_start(w_sb[:], wk[dt])
                    ps0 = psum.tile([128, 512], F32, tag="ps")
                    ps1 = psum.tile([128, 512], F32, tag="ps")
                    for xo in range(XO):
                        nc.tensor.matmul(
                            ps0[:, :SCH],
                            w_sb[:, xo, :],
                            ft_sb[:, xo, 0:SCH],
                            start=(xo == 0), stop=(xo == XO - 1),
                        )
                        nc.tensor.matmul(
                            ps1[:, :SCH],
                            w_sb[:, xo, :],
                            ft_sb[:, xo, SCH:2 * SCH],
                            start=(xo == 0), stop=(xo == XO - 1),
                        )
                    o = a_out.tile([128, P], F32R, tag="ao")
                    nc.vector.tensor_scalar_add(o[:, 0:SCH], ps0[:, :SCH], bk_sb[:, dt:dt + 1])
                    nc.vector.tensor_scalar_add(o[:, SCH:2 * SCH], ps1[:, :SCH], bk_sb[:, dt:dt + 1])
                    nc.sync.dma_start(kt_d[dt], o[:])

            # ---------- Phase B: QT -> qt_dram ----------
            with (
                tc.tile_pool(name="b_w", bufs=3) as b_w,
                tc.tile_pool(name="b_rhs", bufs=1) as b_rhs,
                tc.tile_pool(name="b_out", bufs=3) as b_out,
            ):
                for tcb in range(NTCB):
                    tt_sb = b_rhs.tile([128, XO, TCB], F32R, tag="brhs")
                    nc.sync.dma_start(tt_sb[:], textT_v[:, :, tcb * TCB:(tcb + 1) * TCB])
                    for dt in range(DT):
                        w_sb = b_w.tile([128, XO, 128], F32R, tag="bw")
                        nc.sync.dma_start(w_sb[:], wq[dt])
                        for nb in range(TCB // NB):
                            ps = psum.tile([128, 512], F32, tag="ps")
                            for xo in range(XO):
                                nc.tensor.matmul(
                                    ps[:, :NB],
                                    w_sb[:, xo, :],
                                    tt_sb[:, xo, nb * NB:(nb + 1) * NB],
                                    start=(xo == 0), stop=(xo == XO - 1),
                                )
                            o = b_out.tile([128, NB], F32R, tag="bo")
                            nc.vector.tensor_scalar_add(o[:], ps[:, :NB], bq_sb[:, dt:dt + 1])
                            t0 = tcb * TCB + nb * NB
                            nc.sync.dma_start(qt_d[dt, :, t0:t0 + NB], o[:])

            # ---------- Phase C: scores/softmax/attended ----------
            with (
                tc.tile_pool(name="c_kt", bufs=1) as c_kt,
                tc.tile_pool(name="c_feat", bufs=1) as c_feat,
                tc.tile_pool(name="c_qt", bufs=2) as c_qt,
                tc.tile_pool(name="c_attn", bufs=2) as c_attn,
                tc.tile_pool(name="c_attnT", bufs=2) as c_attnT,
                tc.tile_pool(name="c_stat", bufs=4) as c_stat,
                tc.tile_pool(name="c_out", bufs=3) as c_out,
            ):
                kt_sb = c_kt.tile([128, DT, P], F32R)
                for dt in range(DT):
                    nc.sync.dma_start(kt_sb[:, dt, :], kt_d[dt])
                feat_sb = c_feat.tile([128, PC, D], F32R)
                for pc in range(PC):
                    rows = 128 if pc < PC - 1 else P_LAST
                    nc.sync.dma_start(feat_sb[:rows, pc, :], feat[pc * 128:pc * 128 + rows, :])

                for tt in range(TT):
                    qt_sb = c_qt.tile([128, DT, 128], F32R, tag="cqt")
                    nc.sync.dma_start(
                        qt_sb[:],
                        qt_d[:, :, tt * 128:(tt + 1) * 128].rearrange("dt p t -> p dt t"),
                    )
                    ps0 = psum.tile([128, 512], F32, tag="ps")
                    ps1 = psum.tile([128, 512], F32, tag="ps")
                    for dt in range(DT):
                        nc.tensor.matmul(
                            ps0[:, :SCH],
                            qt_sb[:, dt, :],
                            kt_sb[:, dt, 0:SCH],
                            start=(dt == 0), stop=(dt == DT - 1),
                        )
                        nc.tensor.matmul(
                            ps1[:, :SCH],
                            qt_sb[:, dt, :],
                            kt_sb[:, dt, SCH:2 * SCH],
                            start=(dt == 0), stop=(dt == DT - 1),
                        )
                    # softmax over free dim; normalization deferred to eviction
                    mx0 = c_stat.tile([128, 1], F32, tag="mx0")
                    mx1 = c_stat.tile([128, 1], F32, tag="mx1")
                    nc.vector.tensor_reduce(mx0[:], ps0[:, :SCH], AX, ALU.max)
                    nc.vector.tensor_reduce(mx1[:], ps1[:, :SCH], AX, ALU.max)
                    negmax = c_stat.tile([128, 1], F32, tag="negmax")
                    nc.vector.tensor_tensor(negmax[:], mx0[:], mx1[:], ALU.max)
                    nc.vector.tensor_scalar_mul(negmax[:], negmax[:], -scale)
                    attn = c_attn.tile([128, P], F32, tag="attn")
                    nc.scalar.activation(attn[:, 0:SCH], ps0[:, :SCH], EXP, bias=negmax[:], scale=scale)
                    nc.scalar.activation(attn[:, SCH:2 * SCH], ps1[:, :SCH], EXP, bias=negmax[:], scale=scale)
                    ssum = c_stat.tile([128, 1], F32, tag="ssum")
                    nc.vector.tensor_reduce(ssum[:], attn[:], AX, ALU.add)
                    rsum = c_stat.tile([128, 1], F32, tag="rsum")
                    nc.vector.reciprocal(rsum[:], ssum[:])
                    # transpose attn -> attnT
                    atT = c_attnT.tile([128, PC, 128], F32R, tag="atT")
                    for pc in range(PC):
                        cols = 128 if pc < PC - 1 else P_LAST
                        pst = psum.tile([128, 512], F32, tag="ps")
                        nc.tensor.transpose(pst[:cols, :128], attn[:, pc * 128:pc * 128 + cols], ident[:])
                        nc.vector.tensor_copy(atT[:cols, pc, :], pst[:cols, :128])
                    # attended
                    for dc in range(NDC):
                        pa = psum.tile([128, 512], F32, tag="ps")
                        for pc in range(PC):
                            rows = 128 if pc < PC - 1 else P_LAST
                            nc.tensor.matmul(
                                pa[:, :DC],
                                atT[:rows, pc, :],
                                feat_sb[:rows, pc, dc * DC:(dc + 1) * DC],
                                start=(pc == 0), stop=(pc == PC - 1),
                            )
                        o = c_out.tile([128, DC], F32, tag="co")
                        nc.vector.tensor_scalar_mul(o[:], pa[:, :DC], rsum[:])
                        nc.sync.dma_start(out_v[:, tt, dc * DC:(dc + 1) * DC], o[:])

    nc.compile()
    return nc


def prep_core_inputs(text_i, feat_i, wq_pre, wk_pre, bq_r, bk_r):
    return {
        "textT": np.ascontiguousarray(text_i.T),
        "featT": np.ascontiguousarray(feat_i.T),
        "feat": np.ascontiguousarray(feat_i),
        "wq": wq_pre,
        "wk": wk_pre,
        "bq": bq_r,
        "bk": bk_r,
    }


def prep_weights(Wq, bq, Wk, bk, D=None, X=None):
    D = D or Wq.shape[0]
    X = X or Wq.shape[1]
    DT, XO = D // 128, X // 128
    # w_pre[dt, p, xo, d] = W[dt*128+d, xo*128+p]
    wq_pre = np.ascontiguousarray(
        np.asarray(Wq, np.float32).reshape(DT, 128, XO, 128).transpose(0, 3, 2, 1))
    wk_pre = np.ascontiguousarray(
        np.asarray(Wk, np.float32).reshape(DT, 128, XO, 128).transpose(0, 3, 2, 1))
    bq_r = np.ascontiguousarray(np.asarray(bq, np.float32).reshape(DT, 128).T)
    bk_r = np.ascontiguousarray(np.asarray(bk, np.float32).reshape(DT, 128).T)
    return wq_pre, wk_pre, bq_r, bk_r


_NC_CACHE = {}


def kernel(text, features, Wq, bq, Wk, bk):
    text = np.asarray(text, np.float32)
    features = np.asarray(features, np.float32)
    B, T, X = text.shape
    _, P, _ = features.shape
    D = Wq.shape[0]
    key = (T, P, D, X)
    if key not in _NC_CACHE:
        _NC_CACHE[key] = build_attention_nc(T, P, D, X)
    nc = _NC_CACHE[key]

    wq_pre, wk_pre, bq_r, bk_r = prep_weights(Wq, bq, Wk, bk, D, X)
    in_maps = [
        prep_core_inputs(text[i], features[i], wq_pre, wk_pre, bq_r, bk_r)
        for i in range(B)
    ]
    res = run_bass_kernel_spmd(nc, in_maps, list(range(B)))
    return np.stack([res.results[i]["out"] for i in range(B)], axis=0)
